# revision 20
# baseline (speedup 1.0000x reference)
"""HardNegativeMiningLoss on 8 TRN2 NeuronCores.

Data-parallel over anchor rows: core c owns rows [1024c, 1024(c+1)).
Each core holds full E^T (fp16) in SBUF and computes its [1024, 8192]
sim block with TensorE into 2048-wide PSUM tiles (fp32, half of PSUM,
double buffered).  ScalarE evacuates each 2048-block to fp16 SBUF in
one wide ACT copy.  VectorE applies the semi-hard shift
u - 8*[u >= pos_min] (tensor_scalar is_ge/mult at 4x fp16 rate +
in-place tensor_tensor add at 2x), folds the block 2048->512 with two
packed tensor_tensor max ops, and takes the per-block top-8 with MAX8.
Per row-tile the 4 blocks' top-8s merge to top-16 via
max8/match_replace/max8 and stream out.  Input DMAs are split across
the two hardware DGE queues (SP + Activation) with 512-col slivers for
block 0 so the first matmuls start ~8us in.  The masked logsumexp over
the 16 survivors (0.01% of FLOPs) runs on host in fp64, as does the
label-group metadata (pos_min / pos_sim / valid).
"""

import numpy as np

import concourse.bacc as bacc
import concourse.bass as bass
import concourse.mybir as mybir
import concourse.tile as tile
from concourse.bass_utils import run_bass_kernel_spmd

B = 8192
D = 512
N_CORES = 8
ROWS_PER_CORE = B // N_CORES          # 1024
N_ROW_TILES = ROWS_PER_CORE // 128    # 8
BLK = 2048
N_BLKS = B // BLK                     # 4
SUB = 512                             # psum quarter (one bank)
N_SUBS = BLK // SUB                   # 4
TEMP = 0.07
SHIFT = 8.0
CORR = SHIFT / TEMP
FP = mybir.dt.float32
F16 = mybir.dt.float16
NK = D // 128                         # 4


def _build_program():
    nc = bacc.Bacc(None, target_bir_lowering=False)

    et_d = nc.dram_tensor("et", [D, B], F16, kind="ExternalInput")
    eloc_d = nc.dram_tensor("eloc", [D, ROWS_PER_CORE], F16, kind="ExternalInput")
    meta_d = nc.dram_tensor("rowmeta", [ROWS_PER_CORE, 4], FP, kind="ExternalInput")
    out_d = nc.dram_tensor("out", [N_ROW_TILES, 128, 16], F16,
                           kind="ExternalOutput")

    # permuted views: [partition, k, cols] so one DMA covers all 4 k-tiles
    et_v = et_d[:].rearrange("(k p) n -> p k n", p=128)       # [128,4,B]
    eloc_v = eloc_d[:].rearrange("(k p) n -> p k n", p=128)   # [128,4,1024]
    meta_v = meta_d[:].rearrange("(t p) m -> p t m", p=128)   # [128,8,4]

    with tile.TileContext(nc) as tc:
        with (
            tc.tile_pool(name="wts", bufs=1) as wts,
            tc.tile_pool(name="psum", bufs=2, space="PSUM") as psp,
            tc.tile_pool(name="ub", bufs=4) as ubp,
            tc.tile_pool(name="pen", bufs=4) as penp,
            tc.tile_pool(name="f1", bufs=3) as f1p,
            tc.tile_pool(name="f2", bufs=3) as f2p,
            tc.tile_pool(name="small", bufs=2) as smp,
            tc.tile_pool(name="acc", bufs=1) as accp,
        ):
            # resident inputs: few large multi-k DMAs, ordered so the first
            # row-tile of block 0 can start after ~0.7MB has landed
            metas = accp.tile([128, N_ROW_TILES, 4], FP, tag="metas")
            nc.sync.dma_start(metas[:], meta_v)
            eloc_t = wts.tile([128, NK, ROWS_PER_CORE], F16, tag="eloc")
            nc.sync.dma_start(eloc_t[:, :, 0:128], eloc_v[:, :, 0:128])
            et_t = wts.tile([128, NK, B], F16, tag="et")
            nc.sync.dma_start(et_t[:, :, 0:SUB], et_v[:, :, 0:SUB])
            nc.sync.dma_start(et_t[:, :, SUB:2 * SUB], et_v[:, :, SUB:2 * SUB])
            nc.sync.dma_start(et_t[:, :, 2 * SUB:3 * SUB],
                              et_v[:, :, 2 * SUB:3 * SUB])
            nc.sync.dma_start(et_t[:, :, 3 * SUB:BLK], et_v[:, :, 3 * SUB:BLK])
            nc.sync.dma_start(eloc_t[:, :, 128:ROWS_PER_CORE],
                              eloc_v[:, :, 128:ROWS_PER_CORE])
            for p in range(1, N_BLKS):
                nc.sync.dma_start(et_t[:, :, p * BLK:(p + 1) * BLK],
                                  et_v[:, :, p * BLK:(p + 1) * BLK])

            pool = accp.tile([128, N_ROW_TILES, N_BLKS * 8], F16, tag="pool")
            t16 = accp.tile([128, N_ROW_TILES, 16], F16, tag="t16")

            # block-outer sweeps: all 8 row-tiles on block 0 first, so the
            # later 2048-col pieces have a full sweep (~28us) to arrive
            for blk in range(N_BLKS):
                for rt in range(N_ROW_TILES):
                    pm = metas[:, rt, 0:1]
                    ps = psp.tile([128, BLK], FP, tag="ps")
                    for c in range(N_SUBS):
                        col0 = blk * BLK + c * SUB
                        for k in range(NK):
                            nc.tensor.matmul(
                                ps[:, c * SUB:(c + 1) * SUB],
                                eloc_t[:, k, rt * 128:(rt + 1) * 128],
                                et_t[:, k, col0:col0 + SUB],
                                start=(k == 0),
                                stop=(k == NK - 1),
                            )
                    ub = ubp.tile([128, BLK], F16, tag="ub")
                    nc.scalar.copy(ub[:], ps[:])
                    pen = penp.tile([128, BLK], F16, tag="pen")
                    nc.vector.tensor_scalar(
                        pen[:], ub[:], pm, -SHIFT,
                        op0=mybir.AluOpType.is_ge, op1=mybir.AluOpType.mult,
                    )
                    nc.vector.tensor_tensor(
                        ub[:], ub[:], pen[:], op=mybir.AluOpType.add)
                    f1 = f1p.tile([128, BLK // 2], F16, tag="f1")
                    nc.vector.tensor_tensor(
                        f1[:], ub[:, 0:BLK // 2], ub[:, BLK // 2:BLK],
                        op=mybir.AluOpType.max)
                    f2 = f2p.tile([128, BLK // 4], F16, tag="f2")
                    nc.vector.tensor_tensor(
                        f2[:], f1[:, 0:BLK // 4], f1[:, BLK // 4:BLK // 2],
                        op=mybir.AluOpType.max)
                    f3 = f2p.tile([128, BLK // 8], F16, tag="f3")
                    nc.vector.tensor_tensor(
                        f3[:], f2[:, 0:BLK // 8], f2[:, BLK // 8:BLK // 4],
                        op=mybir.AluOpType.max)
                    nc.vector.max(pool[:, rt, blk * 8:(blk + 1) * 8], f3[:])

                    if blk == N_BLKS - 1:
                        # merge 4 block top-8s -> top-16, stream to host
                        mr = smp.tile([128, N_BLKS * 8], F16, tag="mr")
                        nc.vector.max(t16[:, rt, 0:8], pool[:, rt, :])
                        nc.vector.match_replace(mr[:], t16[:, rt, 0:8],
                                                pool[:, rt, :], -32768.0)
                        nc.vector.max(t16[:, rt, 8:16], mr[:])
                        nc.sync.dma_start(out_d[rt], t16[:, rt, :])

    nc.compile()
    return nc


def _host_rowmeta(emb: np.ndarray, labels: np.ndarray):
    """pos_min / pos_sim / valid per row from label groups (tiny)."""
    Bn = emb.shape[0]
    pos_min = np.full(Bn, 1e30, np.float32)
    pos_sum = np.zeros(Bn, np.float32)
    cnt = np.zeros(Bn, np.int64)
    order = np.argsort(labels, kind="stable")
    sl = labels[order]
    starts = np.flatnonzero(np.r_[True, sl[1:] != sl[:-1]])
    ends = np.r_[starts[1:], Bn]
    for s, e in zip(starts, ends):
        idx = order[s:e]
        n = e - s
        if n < 2:
            continue
        G = emb[idx] @ emb[idx].T          # [n, n] fp32
        np.fill_diagonal(G, np.nan)
        pos_min[idx] = np.nanmin(G, axis=1)
        pos_sum[idx] = np.nansum(G, axis=1)
        cnt[idx] = n - 1
    pos_sim = pos_sum / np.maximum(cnt, 1) / TEMP
    valid = ((cnt > 0) & ((Bn - 1 - cnt) > 0)).astype(np.float32)
    meta = np.zeros((Bn, 4), np.float32)
    meta[:, 0] = pos_min
    meta[:, 1] = pos_sim
    meta[:, 2] = valid
    return meta, valid.sum()


_profile = [None]


def kernel(embeddings: np.ndarray, labels: np.ndarray) -> np.ndarray:
    emb = np.asarray(embeddings, np.float32)
    lab = np.asarray(labels)
    meta, n_valid = _host_rowmeta(emb, lab)

    et = np.ascontiguousarray(emb.T).astype(np.float16)       # [D, B] fp16
    in_maps = []
    for c in range(N_CORES):
        r0 = c * ROWS_PER_CORE
        in_maps.append({
            "et": et,
            "eloc": np.ascontiguousarray(emb[r0:r0 + ROWS_PER_CORE].T)
                      .astype(np.float16),
            "rowmeta": meta[r0:r0 + ROWS_PER_CORE],
        })

    nc = _build_program()
    trace = _profile[0] is not None
    res = run_bass_kernel_spmd(nc, in_maps, list(range(N_CORES)), trace=trace)
    if trace:
        _profile[0] = res

    # host epilogue (fp64): masked logsumexp over the device top-16 per row
    psim = meta[:, 1].astype(np.float64)
    valid = meta[:, 2].astype(np.float64)
    total = 0.0
    for c in range(N_CORES):
        t16 = np.asarray(res.results[c]["out"], np.float64)   # [8,128,16]
        v = t16.reshape(ROWS_PER_CORE, 16)                    # row rt*128+p
        m = v[:, 0]
        hs = (m > -4.0).astype(np.float64)
        se = np.exp((v - m[:, None]) / TEMP).sum(axis=1)
        se2 = np.maximum(se + hs - 1.0, 1e-30)
        lse = m / TEMP + np.log(se2) + (1.0 - hs) * CORR
        r0 = c * ROWS_PER_CORE
        total += np.sum((lse - psim[r0:r0 + ROWS_PER_CORE])
                        * valid[r0:r0 + ROWS_PER_CORE])
    return np.float32(total / max(n_valid, 1.0))


# revision 23
# speedup vs baseline: 1.0124x; 1.0124x over previous
"""HardNegativeMiningLoss on 8 TRN2 NeuronCores.

Data-parallel over anchor rows: core c owns rows [1024c, 1024(c+1)).
Each core holds full E^T (fp16) in SBUF and computes its [1024, 8192]
sim block with TensorE into 2048-wide PSUM tiles (fp32, half of PSUM,
double buffered).  ScalarE evacuates each 2048-block to fp16 SBUF in
one wide ACT copy.  VectorE applies the semi-hard shift
u - 8*[u >= pos_min] (tensor_scalar is_ge/mult at 4x fp16 rate +
in-place tensor_tensor add at 2x), folds the block 2048->256 with
three packed tensor_tensor max ops, and takes the per-block top-8 with
MAX8.  Per row-tile the 4 blocks' top-8s merge to top-16 via
max8/match_replace/max8 and stream out.  The row-tile loop is
block-outer so the later 2048-col E^T pieces have a full sweep to
arrive; block 0 lands as 512-col slivers so the first matmuls start
~14us in.  The masked logsumexp over the 16 survivors (0.01% of
FLOPs) runs on host in fp64, as does the label-group metadata
(pos_min / pos_sim / valid).
"""

import numpy as np

import concourse.bacc as bacc
import concourse.bass as bass
import concourse.mybir as mybir
import concourse.tile as tile
from concourse.bass_utils import run_bass_kernel_spmd

B = 8192
D = 512
N_CORES = 8
ROWS_PER_CORE = B // N_CORES          # 1024
N_ROW_TILES = ROWS_PER_CORE // 128    # 8
BLK = 2048
N_BLKS = B // BLK                     # 4
SUB = 512                             # psum quarter (one bank)
N_SUBS = BLK // SUB                   # 4
TEMP = 0.07
SHIFT = 8.0
CORR = SHIFT / TEMP
FP = mybir.dt.float32
F16 = mybir.dt.float16
NK = D // 128                         # 4


def _build_program():
    nc = bacc.Bacc(None, target_bir_lowering=False)

    et_d = nc.dram_tensor("et", [D, B], F16, kind="ExternalInput")
    eloc_d = nc.dram_tensor("eloc", [D, ROWS_PER_CORE], F16, kind="ExternalInput")
    meta_d = nc.dram_tensor("rowmeta", [ROWS_PER_CORE, 4], FP, kind="ExternalInput")
    out_d = nc.dram_tensor("out", [N_ROW_TILES, 128, 16], F16,
                           kind="ExternalOutput")

    # permuted views: [partition, k, cols] so one DMA covers all 4 k-tiles
    et_v = et_d[:].rearrange("(k p) n -> p k n", p=128)       # [128,4,B]
    eloc_v = eloc_d[:].rearrange("(k p) n -> p k n", p=128)   # [128,4,1024]
    meta_v = meta_d[:].rearrange("(t p) m -> p t m", p=128)   # [128,8,4]

    with tile.TileContext(nc) as tc:
        with (
            tc.tile_pool(name="wts", bufs=1) as wts,
            tc.tile_pool(name="psum", bufs=2, space="PSUM") as psp,
            tc.tile_pool(name="ub", bufs=3) as ubp,
            tc.tile_pool(name="pen", bufs=3) as penp,
            tc.tile_pool(name="f1", bufs=2) as f1p,
            tc.tile_pool(name="f2", bufs=2) as f2p,
            tc.tile_pool(name="small", bufs=2) as smp,
            tc.tile_pool(name="acc", bufs=1) as accp,
        ):
            # resident inputs: few large multi-k DMAs, ordered so the first
            # row-tile of block 0 can start after ~0.7MB has landed
            eloc_t = wts.tile([128, NK, ROWS_PER_CORE], F16, tag="eloc")
            nc.sync.dma_start(eloc_t[:, :, 0:128], eloc_v[:, :, 0:128])
            et_t = wts.tile([128, NK, B], F16, tag="et")
            nc.sync.dma_start(et_t[:, :, 0:SUB], et_v[:, :, 0:SUB])
            metas = accp.tile([128, N_ROW_TILES, 4], FP, tag="metas")
            nc.sync.dma_start(metas[:], meta_v)
            nc.sync.dma_start(et_t[:, :, SUB:2 * SUB], et_v[:, :, SUB:2 * SUB])
            nc.sync.dma_start(et_t[:, :, 2 * SUB:3 * SUB],
                              et_v[:, :, 2 * SUB:3 * SUB])
            nc.sync.dma_start(et_t[:, :, 3 * SUB:BLK], et_v[:, :, 3 * SUB:BLK])
            nc.sync.dma_start(eloc_t[:, :, 128:ROWS_PER_CORE],
                              eloc_v[:, :, 128:ROWS_PER_CORE])
            for p in range(1, N_BLKS):
                nc.sync.dma_start(et_t[:, :, p * BLK:(p + 1) * BLK],
                                  et_v[:, :, p * BLK:(p + 1) * BLK])

            pool = accp.tile([128, N_ROW_TILES, N_BLKS * 8], F16, tag="pool")
            t16 = accp.tile([128, N_ROW_TILES, 16], F16, tag="t16")

            # block-outer sweeps: all 8 row-tiles on block 0 first, so the
            # later 2048-col pieces have a full sweep (~28us) to arrive
            for blk in range(N_BLKS):
                for rt in range(N_ROW_TILES):
                    pm = metas[:, rt, 0:1]
                    ps = psp.tile([128, BLK], FP, tag="ps")
                    for c in range(N_SUBS):
                        col0 = blk * BLK + c * SUB
                        for k in range(NK):
                            nc.tensor.matmul(
                                ps[:, c * SUB:(c + 1) * SUB],
                                eloc_t[:, k, rt * 128:(rt + 1) * 128],
                                et_t[:, k, col0:col0 + SUB],
                                start=(k == 0),
                                stop=(k == NK - 1),
                            )
                    ub = ubp.tile([128, BLK], F16, tag="ub")
                    nc.scalar.copy(ub[:], ps[:])
                    pen = penp.tile([128, BLK], F16, tag="pen")
                    nc.vector.tensor_scalar(
                        pen[:], ub[:], pm, -SHIFT,
                        op0=mybir.AluOpType.is_ge, op1=mybir.AluOpType.mult,
                    )
                    nc.vector.tensor_tensor(
                        ub[:], ub[:], pen[:], op=mybir.AluOpType.add)
                    f1 = f1p.tile([128, BLK // 2], F16, tag="f1")
                    nc.vector.tensor_tensor(
                        f1[:], ub[:, 0:BLK // 2], ub[:, BLK // 2:BLK],
                        op=mybir.AluOpType.max)
                    f2 = f2p.tile([128, BLK // 4], F16, tag="f2")
                    nc.vector.tensor_tensor(
                        f2[:], f1[:, 0:BLK // 4], f1[:, BLK // 4:BLK // 2],
                        op=mybir.AluOpType.max)
                    f3 = f2p.tile([128, BLK // 8], F16, tag="f3")
                    nc.vector.tensor_tensor(
                        f3[:], f2[:, 0:BLK // 8], f2[:, BLK // 8:BLK // 4],
                        op=mybir.AluOpType.max)
                    nc.vector.max(pool[:, rt, blk * 8:(blk + 1) * 8], f3[:])

                    if blk == N_BLKS - 1:
                        # merge 4 block top-8s -> top-16, stream to host
                        mr = smp.tile([128, N_BLKS * 8], F16, tag="mr")
                        nc.vector.max(t16[:, rt, 0:8], pool[:, rt, :])
                        nc.vector.match_replace(mr[:], t16[:, rt, 0:8],
                                                pool[:, rt, :], -32768.0)
                        nc.vector.max(t16[:, rt, 8:16], mr[:])
                        nc.sync.dma_start(out_d[rt], t16[:, rt, :])

    nc.compile()
    return nc


def _host_rowmeta(emb: np.ndarray, labels: np.ndarray):
    """pos_min / pos_sim / valid per row from label groups (tiny)."""
    Bn = emb.shape[0]
    pos_min = np.full(Bn, 1e30, np.float32)
    pos_sum = np.zeros(Bn, np.float32)
    cnt = np.zeros(Bn, np.int64)
    order = np.argsort(labels, kind="stable")
    sl = labels[order]
    starts = np.flatnonzero(np.r_[True, sl[1:] != sl[:-1]])
    ends = np.r_[starts[1:], Bn]
    for s, e in zip(starts, ends):
        idx = order[s:e]
        n = e - s
        if n < 2:
            continue
        G = emb[idx] @ emb[idx].T          # [n, n] fp32
        np.fill_diagonal(G, np.nan)
        pos_min[idx] = np.nanmin(G, axis=1)
        pos_sum[idx] = np.nansum(G, axis=1)
        cnt[idx] = n - 1
    pos_sim = pos_sum / np.maximum(cnt, 1) / TEMP
    valid = ((cnt > 0) & ((Bn - 1 - cnt) > 0)).astype(np.float32)
    meta = np.zeros((Bn, 4), np.float32)
    meta[:, 0] = pos_min
    meta[:, 1] = pos_sim
    meta[:, 2] = valid
    return meta, valid.sum()


_profile = [None]


def kernel(embeddings: np.ndarray, labels: np.ndarray) -> np.ndarray:
    emb = np.asarray(embeddings, np.float32)
    lab = np.asarray(labels)
    meta, n_valid = _host_rowmeta(emb, lab)

    et = np.ascontiguousarray(emb.T).astype(np.float16)       # [D, B] fp16
    in_maps = []
    for c in range(N_CORES):
        r0 = c * ROWS_PER_CORE
        in_maps.append({
            "et": et,
            "eloc": np.ascontiguousarray(emb[r0:r0 + ROWS_PER_CORE].T)
                      .astype(np.float16),
            "rowmeta": meta[r0:r0 + ROWS_PER_CORE],
        })

    nc = _build_program()
    trace = _profile[0] is not None
    res = run_bass_kernel_spmd(nc, in_maps, list(range(N_CORES)), trace=trace)
    if trace:
        _profile[0] = res

    # host epilogue (fp64): masked logsumexp over the device top-16 per row
    psim = meta[:, 1].astype(np.float64)
    valid = meta[:, 2].astype(np.float64)
    total = 0.0
    for c in range(N_CORES):
        t16 = np.asarray(res.results[c]["out"], np.float64)   # [8,128,16]
        v = t16.reshape(ROWS_PER_CORE, 16)                    # row rt*128+p
        m = v[:, 0]
        hs = (m > -4.0).astype(np.float64)
        se = np.exp((v - m[:, None]) / TEMP).sum(axis=1)
        se2 = np.maximum(se + hs - 1.0, 1e-30)
        lse = m / TEMP + np.log(se2) + (1.0 - hs) * CORR
        r0 = c * ROWS_PER_CORE
        total += np.sum((lse - psim[r0:r0 + ROWS_PER_CORE])
                        * valid[r0:r0 + ROWS_PER_CORE])
    return np.float32(total / max(n_valid, 1.0))


# revision 31
# speedup vs baseline: 1.0227x; 1.0103x over previous
"""HardNegativeMiningLoss on 8 TRN2 NeuronCores.

Data-parallel over anchor rows: core c owns rows [1024c, 1024(c+1)).
Each core holds full E^T (fp16) in SBUF and computes its [1024, 8192]
sim block with TensorE into 2048-wide PSUM tiles (fp32, half of PSUM,
double buffered).  ScalarE evacuates each 2048-block to fp16 SBUF in
one wide ACT copy.  VectorE applies the semi-hard shift
u - 8*[u >= pos_min] (tensor_scalar is_ge/mult at 4x fp16 rate +
in-place tensor_tensor add at 2x), folds the block 2048->256 with
three packed tensor_tensor max ops, and takes the per-block top-8 with
MAX8.  Per row-tile the 4 blocks' top-8s merge to top-16 via
max8/match_replace/max8 and stream out.  The row-tile loop is
block-outer so the later 2048-col E^T pieces have a full sweep to
arrive; block 0 lands as 512-col slivers so the first matmuls start
~14us in.  The masked logsumexp over the 16 survivors (0.01% of
FLOPs) runs on host in fp64, as does the label-group metadata
(pos_min / pos_sim / valid).
"""

import numpy as np

import concourse.bacc as bacc
import concourse.bass as bass
import concourse.mybir as mybir
import concourse.tile as tile
from concourse.bass_utils import run_bass_kernel_spmd

B = 8192
D = 512
N_CORES = 8
ROWS_PER_CORE = B // N_CORES          # 1024
N_ROW_TILES = ROWS_PER_CORE // 128    # 8
BLK = 2048
N_BLKS = B // BLK                     # 4
SUB = 512                             # psum quarter (one bank)
N_SUBS = BLK // SUB                   # 4
TEMP = 0.07
SHIFT = 8.0
CORR = SHIFT / TEMP
FP = mybir.dt.float32
F16 = mybir.dt.float16
NK = D // 128                         # 4


def _build_program():
    nc = bacc.Bacc(None, target_bir_lowering=False)

    et_d = nc.dram_tensor("et", [D, B], F16, kind="ExternalInput")
    eloc_d = nc.dram_tensor("eloc", [D, ROWS_PER_CORE], F16, kind="ExternalInput")
    meta_d = nc.dram_tensor("rowmeta", [ROWS_PER_CORE, 4], FP, kind="ExternalInput")
    out_d = nc.dram_tensor("out", [N_ROW_TILES, 128, 32], F16,
                           kind="ExternalOutput")

    # permuted views: [partition, k, cols] so one DMA covers all 4 k-tiles
    et_v = et_d[:].rearrange("(k p) n -> p k n", p=128)       # [128,4,B]
    eloc_v = eloc_d[:].rearrange("(k p) n -> p k n", p=128)   # [128,4,1024]
    meta_v = meta_d[:].rearrange("(t p) m -> p t m", p=128)   # [128,8,4]

    with tile.TileContext(nc) as tc:
        with (
            tc.tile_pool(name="wts", bufs=1) as wts,
            tc.tile_pool(name="psum", bufs=2, space="PSUM") as psp,
            tc.tile_pool(name="ub", bufs=3) as ubp,
            tc.tile_pool(name="pen", bufs=3) as penp,
            tc.tile_pool(name="f1", bufs=2) as f1p,
            tc.tile_pool(name="f2", bufs=2) as f2p,
            tc.tile_pool(name="acc", bufs=1) as accp,
        ):
            # resident inputs: few large multi-k DMAs, ordered so the first
            # row-tile of block 0 can start after ~0.7MB has landed
            eloc_t = wts.tile([128, NK, ROWS_PER_CORE], F16, tag="eloc")
            nc.sync.dma_start(eloc_t[:, :, 0:128], eloc_v[:, :, 0:128])
            et_t = wts.tile([128, NK, B], F16, tag="et")
            nc.sync.dma_start(et_t[:, :, 0:SUB], et_v[:, :, 0:SUB])
            metas = accp.tile([128, N_ROW_TILES, 4], FP, tag="metas")
            nc.sync.dma_start(metas[:], meta_v)
            nc.sync.dma_start(et_t[:, :, SUB:BLK], et_v[:, :, SUB:BLK])
            nc.sync.dma_start(eloc_t[:, :, 128:ROWS_PER_CORE],
                              eloc_v[:, :, 128:ROWS_PER_CORE])
            for p in range(1, N_BLKS):
                nc.sync.dma_start(et_t[:, :, p * BLK:(p + 1) * BLK],
                                  et_v[:, :, p * BLK:(p + 1) * BLK])

            pool = accp.tile([128, N_ROW_TILES, N_BLKS * 8], F16, tag="pool")

            # block-outer sweeps: all 8 row-tiles on block 0 first, so the
            # later 2048-col pieces have a full sweep (~28us) to arrive
            for blk in range(N_BLKS):
                for rt in range(N_ROW_TILES):
                    pm = metas[:, rt, 0:1]
                    ps = psp.tile([128, BLK], FP, tag="ps")
                    for c in range(N_SUBS):
                        col0 = blk * BLK + c * SUB
                        for k in range(NK):
                            nc.tensor.matmul(
                                ps[:, c * SUB:(c + 1) * SUB],
                                eloc_t[:, k, rt * 128:(rt + 1) * 128],
                                et_t[:, k, col0:col0 + SUB],
                                start=(k == 0),
                                stop=(k == NK - 1),
                            )
                    ub = ubp.tile([128, BLK], F16, tag="ub")
                    nc.scalar.copy(ub[:], ps[:])
                    pen = penp.tile([128, BLK], F16, tag="pen")
                    nc.vector.tensor_scalar(
                        pen[:], ub[:], pm, -SHIFT,
                        op0=mybir.AluOpType.is_ge, op1=mybir.AluOpType.mult,
                    )
                    nc.vector.tensor_tensor(
                        ub[:], ub[:], pen[:], op=mybir.AluOpType.add)
                    f1 = f1p.tile([128, BLK // 2], F16, tag="f1")
                    nc.vector.tensor_tensor(
                        f1[:], ub[:, 0:BLK // 2], ub[:, BLK // 2:BLK],
                        op=mybir.AluOpType.max)
                    f2 = f2p.tile([128, BLK // 4], F16, tag="f2")
                    nc.vector.tensor_tensor(
                        f2[:], f1[:, 0:BLK // 4], f1[:, BLK // 4:BLK // 2],
                        op=mybir.AluOpType.max)
                    f3 = f2p.tile([128, BLK // 8], F16, tag="f3")
                    nc.vector.tensor_tensor(
                        f3[:], f2[:, 0:BLK // 8], f2[:, BLK // 8:BLK // 4],
                        op=mybir.AluOpType.max)
                    nc.vector.max(pool[:, rt, blk * 8:(blk + 1) * 8], f3[:])

                    if blk == N_BLKS - 1:
                        # stream the raw 32-candidate pool; host merges to
                        # top-16 (bit-identical to a device-side merge)
                        nc.sync.dma_start(out_d[rt], pool[:, rt, :])

    nc.compile()
    return nc


def _host_rowmeta(emb: np.ndarray, labels: np.ndarray):
    """pos_min / pos_sim / valid per row from label groups (tiny)."""
    Bn = emb.shape[0]
    pos_min = np.full(Bn, 1e30, np.float32)
    pos_sum = np.zeros(Bn, np.float32)
    cnt = np.zeros(Bn, np.int64)
    order = np.argsort(labels, kind="stable")
    sl = labels[order]
    starts = np.flatnonzero(np.r_[True, sl[1:] != sl[:-1]])
    ends = np.r_[starts[1:], Bn]
    for s, e in zip(starts, ends):
        idx = order[s:e]
        n = e - s
        if n < 2:
            continue
        G = emb[idx] @ emb[idx].T          # [n, n] fp32
        np.fill_diagonal(G, np.nan)
        pos_min[idx] = np.nanmin(G, axis=1)
        pos_sum[idx] = np.nansum(G, axis=1)
        cnt[idx] = n - 1
    pos_sim = pos_sum / np.maximum(cnt, 1) / TEMP
    valid = ((cnt > 0) & ((Bn - 1 - cnt) > 0)).astype(np.float32)
    meta = np.zeros((Bn, 4), np.float32)
    meta[:, 0] = pos_min
    meta[:, 1] = pos_sim
    meta[:, 2] = valid
    return meta, valid.sum()


_profile = [None]


def kernel(embeddings: np.ndarray, labels: np.ndarray) -> np.ndarray:
    emb = np.asarray(embeddings, np.float32)
    lab = np.asarray(labels)
    meta, n_valid = _host_rowmeta(emb, lab)

    et = np.ascontiguousarray(emb.T).astype(np.float16)       # [D, B] fp16
    in_maps = []
    for c in range(N_CORES):
        r0 = c * ROWS_PER_CORE
        in_maps.append({
            "et": et,
            "eloc": np.ascontiguousarray(emb[r0:r0 + ROWS_PER_CORE].T)
                      .astype(np.float16),
            "rowmeta": meta[r0:r0 + ROWS_PER_CORE],
        })

    nc = _build_program()
    trace = _profile[0] is not None
    res = run_bass_kernel_spmd(nc, in_maps, list(range(N_CORES)), trace=trace)
    if trace:
        _profile[0] = res

    # host epilogue (fp64): masked logsumexp over the device top-16 per row
    psim = meta[:, 1].astype(np.float64)
    valid = meta[:, 2].astype(np.float64)
    total = 0.0
    for c in range(N_CORES):
        p32 = np.asarray(res.results[c]["out"], np.float64)   # [8,128,32]
        v = -np.sort(-p32.reshape(ROWS_PER_CORE, 32), axis=1)[:, :16]
        m = v[:, 0]
        hs = (m > -4.0).astype(np.float64)
        se = np.exp((v - m[:, None]) / TEMP).sum(axis=1)
        se2 = np.maximum(se + hs - 1.0, 1e-30)
        lse = m / TEMP + np.log(se2) + (1.0 - hs) * CORR
        r0 = c * ROWS_PER_CORE
        total += np.sum((lse - psim[r0:r0 + ROWS_PER_CORE])
                        * valid[r0:r0 + ROWS_PER_CORE])
    return np.float32(total / max(n_valid, 1.0))
